# Initial kernel scaffold
#
"""Trainium2 Bass kernel for CustomPatchEmbedding (ragged patch gather + two projections).

Strategy (data-parallel over batch, 8 cores x 4 images):
  - Patch pixel rows are gathered straight from HBM images via SWDGE
    indirect DMA (one descriptor per contiguous patch row), landing as
    [patch, feature] tiles in SBUF with features in (c, dy, dx) order.
  - Gather indices are computed on-chip from the xy tensors (shift/add on
    DVE) plus small constant offset tables supplied as inputs.
  - TensorE transposes each 128-feature chunk to [feature, patch], then
    accumulates lhsT.T @ W^T chunks into PSUM ([patch, 256] fp32).
  - Bias is added from a partition-replicated bias tile; results DMA to DRAM.

kernel(**inputs) takes the FULL unsharded inputs and returns (32, 288, 256) f32.
"""
import sys
import numpy as np

sys.path.insert(0, "/opt/trn_rl_repo")

import concourse.bass as bass
import concourse.bacc as bacc
import concourse.mybir as mybir
import concourse.tile as tile
from concourse.masks import make_identity
from concourse.bass_utils import run_bass_kernel_spmd
from contextlib import ExitStack

# Problem constants (hardcoded per spec).
B, C, H, W = 32, 3, 512, 512
FP, CP = 16, 64
NF, NCO = 256, 32
D = 256
NCORES = 8
IPC = B // NCORES              # images per core
CHW = C * H * W                # 786432, per-image flat element count
NFLAT = IPC * CHW              # flat image elements per core
KF = C * FP * FP               # 768  fine features
KC = C * CP * CP               # 12288 coarse features
NROW_F = C * FP                # 48 gather rows per fine patch (c,dy)
NROW_C = C * CP                # 192 gather rows per coarse patch
P = 128

FDT = mybir.dt.float32
IDT = mybir.dt.int32

# Coarse gather is split into column-chunks of the index tile.
CJ = 24                        # idx columns per coarse gather chunk
NCHUNK_C = NROW_C // CJ        # 8 chunks
KPC = CJ * CP // P             # k-chunks (of 128) per coarse gather chunk = 12
NKF = KF // P                  # 6 fine k-chunks
NKC = KC // P                  # 96 coarse k-chunks


import os
VARIANT = os.environ.get("KVARIANT", "full")  # full | nogather | gatheronly


def _emit(nc, tc, t):
    """Emit the per-core Tile program. `t` maps tensor name -> dram handle."""
    no_gather = VARIANT == "nogather"
    gather_only = VARIANT == "gatheronly"
    with ExitStack() as ctx:
        const = ctx.enter_context(tc.tile_pool(name="const", bufs=1))
        small = ctx.enter_context(tc.tile_pool(name="small", bufs=1))
        gf_pool = ctx.enter_context(tc.tile_pool(name="gf", bufs=3))
        gc_pool = ctx.enter_context(tc.tile_pool(name="gc", bufs=3))
        wf_pool = ctx.enter_context(tc.tile_pool(name="wf", bufs=6))
        wc_pool = ctx.enter_context(tc.tile_pool(name="wc", bufs=14))
        lt_pool = ctx.enter_context(tc.tile_pool(name="lt", bufs=14))
        ob_pool = ctx.enter_context(tc.tile_pool(name="ob", bufs=3))
        ps_tp = ctx.enter_context(tc.tile_pool(name="ps_tp", bufs=4, space="PSUM"))
        ps_f = ctx.enter_context(tc.tile_pool(name="ps_f", bufs=2, space="PSUM"))
        ps_c = ctx.enter_context(tc.tile_pool(name="ps_c", bufs=1, space="PSUM"))

        # --- constants ---
        identity = const.tile([P, P], FDT)
        make_identity(nc, identity[:])
        tbl_f = const.tile([P, NROW_F], IDT)
        nc.sync.dma_start(tbl_f[:], t["tbl_f"][:])
        tbl_c = const.tile([P, NROW_C], IDT)
        nc.sync.dma_start(tbl_c[:], t["tbl_c"][:])
        bias_f = const.tile([P, D], FDT)
        nc.sync.dma_start(bias_f[:], t["bias_f"][:])
        bias_c = const.tile([P, D], FDT)
        nc.sync.dma_start(bias_c[:], t["bias_c"][:])

        # --- gather indices ---
        # coarse: one [128, 192] tile; partition p = (img, patch), col j = (c, dy)
        cxy = small.tile([P, 2], IDT)
        nc.sync.dma_start(cxy[:], t["coarse_xy"].ap().rearrange("b n two -> (b n) two"))
        cbase = small.tile([P, 1], IDT)
        nc.vector.tensor_scalar(
            out=cbase[:], in0=cxy[:, 1:2], scalar1=9, scalar2=None,
            op0=mybir.AluOpType.logical_shift_left,
        )
        nc.vector.tensor_tensor(
            out=cbase[:], in0=cbase[:], in1=cxy[:, 0:1], op=mybir.AluOpType.add
        )
        cidx = small.tile([P, NROW_C], IDT)
        nc.vector.tensor_tensor(
            out=cidx[:], in0=tbl_c[:], in1=cbase[:].to_broadcast([P, NROW_C]),
            op=mybir.AluOpType.add,
        )

        # fine: per (img b, half h) a [128, 48] tile
        fidx = []
        for b in range(IPC):
            for h in range(2):
                fxy = small.tile([P, 2], IDT, tag="fxy")
                nc.sync.dma_start(fxy[:], t["fine_xy"][b, h * P:(h + 1) * P, :])
                fb = small.tile([P, 1], IDT, tag="fb")
                nc.vector.tensor_scalar(
                    out=fb[:], in0=fxy[:, 1:2], scalar1=9, scalar2=None,
                    op0=mybir.AluOpType.logical_shift_left,
                )
                nc.vector.tensor_tensor(
                    out=fb[:], in0=fb[:], in1=fxy[:, 0:1], op=mybir.AluOpType.add
                )
                nc.vector.tensor_scalar(
                    out=fb[:], in0=fb[:], scalar1=b * CHW, scalar2=None,
                    op0=mybir.AluOpType.add,
                )
                fi = small.tile([P, NROW_F], IDT, tag=f"fidx{b}{h}")
                nc.vector.tensor_tensor(
                    out=fi[:], in0=tbl_f[:], in1=fb[:].to_broadcast([P, NROW_F]),
                    op=mybir.AluOpType.add,
                )
                fidx.append(fi)

        images = t["images"]
        out = t["out"]

        # --- fine branch: 8 groups of 128 patches ---
        # HW indirect DMA consumes ONE offset per destination partition, so each
        # (c,dy) row-column is its own gather instruction writing a 16-elem slice.
        for g in range(IPC * 2):
            b, h = divmod(g, 2)
            gt = gf_pool.tile([P, KF], FDT)
            if no_gather:
                nc.sync.dma_start(
                    gt[:],
                    images.ap().rearrange("(a f) one -> a (f one)", f=KF)[g * P:(g + 1) * P, :],
                )
            else:
                for j in range(NROW_F):
                    nc.gpsimd.indirect_dma_start(
                        out=gt[:, j * FP:(j + 1) * FP], out_offset=None, in_=images[:],
                        in_offset=bass.IndirectOffsetOnAxis(ap=fidx[g][:, j:j + 1], axis=0),
                    )
            if gather_only:
                nc.sync.dma_start(
                    out[b * (NF + NCO) + h * P:b * (NF + NCO) + (h + 1) * P, :],
                    gt[:, :D],
                )
                continue
            psum = ps_f.tile([P, D], FDT)
            lts = []
            for k in range(NKF):
                tp = ps_tp.tile([P, P], FDT, tag="tp")
                nc.tensor.transpose(
                    out=tp[:], in_=gt[:, k * P:(k + 1) * P], identity=identity[:]
                )
                lt = lt_pool.tile([P, P], FDT, tag="lt")
                nc.vector.tensor_copy(lt[:], tp[:])
                lts.append(lt)
            for k in range(NKF):
                wk = wf_pool.tile([P, D], FDT, tag="wf")
                nc.sync.dma_start(wk[:], t["wfT"][k * P:(k + 1) * P, :])
                nc.tensor.matmul(
                    out=psum[:], lhsT=lts[k][:], rhs=wk[:],
                    start=(k == 0), stop=(k == NKF - 1),
                )
            ob = ob_pool.tile([P, D], FDT, tag="ob")
            nc.vector.tensor_tensor(
                out=ob[:], in0=psum[:], in1=bias_f[:], op=mybir.AluOpType.add
            )
            nc.sync.dma_start(out[b * (NF + NCO) + h * P:b * (NF + NCO) + (h + 1) * P, :], ob[:])

        # --- coarse branch: one group of 128 patches, 96 k-chunks ---
        psum_c = None if gather_only else ps_c.tile([P, D], FDT)
        for cc in range(NCHUNK_C):
            gt = gc_pool.tile([P, CJ * CP], FDT)
            if no_gather:
                nc.sync.dma_start(
                    gt[:],
                    images.ap().rearrange("(a f) one -> a (f one)", f=CJ * CP)[cc * P:(cc + 1) * P, :],
                )
            else:
                for j in range(CJ):
                    nc.gpsimd.indirect_dma_start(
                        out=gt[:, j * CP:(j + 1) * CP], out_offset=None, in_=images[:],
                        in_offset=bass.IndirectOffsetOnAxis(
                            ap=cidx[:, cc * CJ + j:cc * CJ + j + 1], axis=0
                        ),
                    )
            if gather_only:
                nc.sync.dma_start(out[cc * P:(cc + 1) * P, :], gt[:, :D])
                continue
            lts = []
            for kk in range(KPC):
                tp = ps_tp.tile([P, P], FDT, tag="tp")
                nc.tensor.transpose(
                    out=tp[:], in_=gt[:, kk * P:(kk + 1) * P], identity=identity[:]
                )
                lt = lt_pool.tile([P, P], FDT, tag="lt")
                nc.vector.tensor_copy(lt[:], tp[:])
                lts.append(lt)
            for kk in range(KPC):
                k = cc * KPC + kk
                wk = wc_pool.tile([P, D], FDT, tag="wc")
                nc.sync.dma_start(wk[:], t["wcT"][k * P:(k + 1) * P, :])
                nc.tensor.matmul(
                    out=psum_c[:], lhsT=lts[kk][:], rhs=wk[:],
                    start=(k == 0), stop=(k == NKC - 1),
                )
        if not gather_only:
            oc = ob_pool.tile([P, D], FDT, tag="oc")
            nc.vector.tensor_tensor(
                out=oc[:], in0=psum_c[:], in1=bias_c[:], op=mybir.AluOpType.add
            )
            for b in range(IPC):
                nc.sync.dma_start(
                    out[b * (NF + NCO) + NF:b * (NF + NCO) + NF + NCO, :],
                    oc[b * NCO:(b + 1) * NCO, :],
                )


def build(reps: int = 1):
    nc = bacc.Bacc("TRN2", target_bir_lowering=False, debug=False)
    t = {
        "images": nc.dram_tensor("images", [NFLAT, 1], FDT, kind="ExternalInput"),
        "fine_xy": nc.dram_tensor("fine_xy", [IPC, NF, 2], IDT, kind="ExternalInput"),
        "coarse_xy": nc.dram_tensor("coarse_xy", [IPC, NCO, 2], IDT, kind="ExternalInput"),
        "wfT": nc.dram_tensor("wfT", [KF, D], FDT, kind="ExternalInput"),
        "wcT": nc.dram_tensor("wcT", [KC, D], FDT, kind="ExternalInput"),
        "bias_f": nc.dram_tensor("bias_f", [P, D], FDT, kind="ExternalInput"),
        "bias_c": nc.dram_tensor("bias_c", [P, D], FDT, kind="ExternalInput"),
        "tbl_f": nc.dram_tensor("tbl_f", [P, NROW_F], IDT, kind="ExternalInput"),
        "tbl_c": nc.dram_tensor("tbl_c", [P, NROW_C], IDT, kind="ExternalInput"),
        "out": nc.dram_tensor("out", [IPC * (NF + NCO), D], FDT, kind="ExternalOutput"),
    }
    with tile.TileContext(nc) as tc:
        for _ in range(reps):
            _emit(nc, tc, t)
    nc.compile()
    return nc


def host_tables():
    jf = np.arange(NROW_F)
    tbl_f = ((jf // FP) * H * W + (jf % FP) * W).astype(np.int32)
    tbl_f = np.repeat(tbl_f[None, :], P, axis=0)
    pc = np.arange(P)[:, None]
    jc = np.arange(NROW_C)[None, :]
    tbl_c = ((pc // NCO) * CHW + (jc // CP) * H * W + (jc % CP) * W).astype(np.int32)
    tbl_c = np.ascontiguousarray(tbl_c)
    return tbl_f, tbl_c


def make_in_maps(images, W_fine, b_fine, W_coarse, b_coarse, fine_xy, coarse_xy):
    images = np.asarray(images, dtype=np.float32)
    fine_xy = np.asarray(fine_xy, dtype=np.int32)
    coarse_xy = np.asarray(coarse_xy, dtype=np.int32)
    wfT = np.ascontiguousarray(np.asarray(W_fine, dtype=np.float32).T)
    wcT = np.ascontiguousarray(np.asarray(W_coarse, dtype=np.float32).T)
    bias_f = np.ascontiguousarray(np.repeat(np.asarray(b_fine, np.float32)[None, :], P, axis=0))
    bias_c = np.ascontiguousarray(np.repeat(np.asarray(b_coarse, np.float32)[None, :], P, axis=0))
    tbl_f, tbl_c = host_tables()
    in_maps = []
    for c in range(NCORES):
        sl = slice(c * IPC, (c + 1) * IPC)
        in_maps.append({
            "images": np.ascontiguousarray(images[sl]).reshape(NFLAT, 1),
            "fine_xy": np.ascontiguousarray(fine_xy[sl]),
            "coarse_xy": np.ascontiguousarray(coarse_xy[sl]),
            "wfT": wfT, "wcT": wcT,
            "bias_f": bias_f, "bias_c": bias_c,
            "tbl_f": tbl_f, "tbl_c": tbl_c,
        })
    return in_maps


_NC_CACHE = []


def _get_nc():
    if not _NC_CACHE:
        _NC_CACHE.append(build())
    return _NC_CACHE[0]


def run(inputs: dict, trace: bool = False):
    nc = _get_nc()
    in_maps = make_in_maps(**inputs)
    res = run_bass_kernel_spmd(nc, in_maps, list(range(NCORES)), trace=trace)
    outs = [
        np.asarray(res.results[c]["out"]).reshape(IPC, NF + NCO, D)
        for c in range(NCORES)
    ]
    return np.concatenate(outs, axis=0), res


def kernel(**inputs) -> np.ndarray:
    out, _ = run(inputs, trace=False)
    return out



# revision 1
# speedup vs baseline: 1.0358x; 1.0358x over previous
"""Trainium2 Bass kernel for CustomPatchEmbedding (ragged patch gather + two projections).

Strategy (data-parallel over batch, 8 cores x 4 images):
  - Patch pixel rows are gathered straight from HBM images via SWDGE
    indirect DMA (one descriptor per contiguous patch row), landing as
    [patch, feature] tiles in SBUF with features in (c, dy, dx) order.
  - Gather indices are computed on-chip from the xy tensors (shift/add on
    DVE) plus small constant offset tables supplied as inputs.
  - TensorE transposes each 128-feature chunk to [feature, patch], then
    accumulates lhsT.T @ W^T chunks into PSUM ([patch, 256] fp32).
  - Bias is added from a partition-replicated bias tile; results DMA to DRAM.

kernel(**inputs) takes the FULL unsharded inputs and returns (32, 288, 256) f32.
"""
import sys
import numpy as np

sys.path.insert(0, "/opt/trn_rl_repo")

import concourse.bass as bass
import concourse.bacc as bacc
import concourse.mybir as mybir
import concourse.tile as tile
from concourse.masks import make_identity
from concourse.bass_utils import run_bass_kernel_spmd
from contextlib import ExitStack

# Problem constants (hardcoded per spec).
B, C, H, W = 32, 3, 512, 512
FP, CP = 16, 64
NF, NCO = 256, 32
D = 256
NCORES = 8
IPC = B // NCORES              # images per core
CHW = C * H * W                # 786432, per-image flat element count
NFLAT = IPC * CHW              # flat image elements per core
KF = C * FP * FP               # 768  fine features
KC = C * CP * CP               # 12288 coarse features
NROW_F = C * FP                # 48 gather rows per fine patch (c,dy)
NROW_C = C * CP                # 192 gather rows per coarse patch
P = 128

FDT = mybir.dt.float32
IDT = mybir.dt.int32

# Coarse gather is split into column-chunks of the index tile.
CJ = 24                        # idx columns per coarse gather chunk
NCHUNK_C = NROW_C // CJ        # 8 chunks
KPC = CJ * CP // P             # k-chunks (of 128) per coarse gather chunk = 12
NKF = KF // P                  # 6 fine k-chunks
NKC = KC // P                  # 96 coarse k-chunks


import os
VARIANT = os.environ.get("KVARIANT", "full")  # full | nogather | gatheronly


def _emit(nc, tc, t):
    """Emit the per-core Tile program. `t` maps tensor name -> dram handle."""
    no_gather = VARIANT == "nogather"
    gather_only = VARIANT == "gatheronly"
    with ExitStack() as ctx:
        const = ctx.enter_context(tc.tile_pool(name="const", bufs=1))
        small = ctx.enter_context(tc.tile_pool(name="small", bufs=1))
        gf_pool = ctx.enter_context(tc.tile_pool(name="gf", bufs=3))
        gc_pool = ctx.enter_context(tc.tile_pool(name="gc", bufs=3))
        wf_pool = ctx.enter_context(tc.tile_pool(name="wf", bufs=6))
        wc_pool = ctx.enter_context(tc.tile_pool(name="wc", bufs=14))
        lt_pool = ctx.enter_context(tc.tile_pool(name="lt", bufs=14))
        ob_pool = ctx.enter_context(tc.tile_pool(name="ob", bufs=3))
        ps_tp = ctx.enter_context(tc.tile_pool(name="ps_tp", bufs=4, space="PSUM"))
        ps_f = ctx.enter_context(tc.tile_pool(name="ps_f", bufs=2, space="PSUM"))
        ps_c = ctx.enter_context(tc.tile_pool(name="ps_c", bufs=1, space="PSUM"))

        # --- constants ---
        identity = const.tile([P, P], FDT)
        make_identity(nc, identity[:])
        tbl_f = const.tile([P, NROW_F], IDT)
        nc.sync.dma_start(tbl_f[:], t["tbl_f"][:])
        tbl_c = const.tile([P, NROW_C], IDT)
        nc.sync.dma_start(tbl_c[:], t["tbl_c"][:])
        bias_f = const.tile([P, D], FDT)
        nc.sync.dma_start(bias_f[:], t["bias_f"][:])
        bias_c = const.tile([P, D], FDT)
        nc.sync.dma_start(bias_c[:], t["bias_c"][:])

        # --- gather indices ---
        # coarse: one [128, 192] tile; partition p = (img, patch), col j = (c, dy)
        cxy = small.tile([P, 2], IDT)
        nc.sync.dma_start(cxy[:], t["coarse_xy"].ap().rearrange("b n two -> (b n) two"))
        cbase = small.tile([P, 1], IDT)
        nc.vector.tensor_scalar(
            out=cbase[:], in0=cxy[:, 1:2], scalar1=9, scalar2=None,
            op0=mybir.AluOpType.logical_shift_left,
        )
        nc.vector.tensor_tensor(
            out=cbase[:], in0=cbase[:], in1=cxy[:, 0:1], op=mybir.AluOpType.add
        )
        cidx = small.tile([P, NROW_C], IDT)
        nc.vector.tensor_tensor(
            out=cidx[:], in0=tbl_c[:], in1=cbase[:].to_broadcast([P, NROW_C]),
            op=mybir.AluOpType.add,
        )

        # fine: per (img b, half h) a [128, 48] tile
        fidx = []
        for b in range(IPC):
            for h in range(2):
                fxy = small.tile([P, 2], IDT, tag="fxy")
                nc.sync.dma_start(fxy[:], t["fine_xy"][b, h * P:(h + 1) * P, :])
                fb = small.tile([P, 1], IDT, tag="fb")
                nc.vector.tensor_scalar(
                    out=fb[:], in0=fxy[:, 1:2], scalar1=9, scalar2=None,
                    op0=mybir.AluOpType.logical_shift_left,
                )
                nc.vector.tensor_tensor(
                    out=fb[:], in0=fb[:], in1=fxy[:, 0:1], op=mybir.AluOpType.add
                )
                nc.vector.tensor_scalar(
                    out=fb[:], in0=fb[:], scalar1=b * CHW, scalar2=None,
                    op0=mybir.AluOpType.add,
                )
                fi = small.tile([P, NROW_F], IDT, tag=f"fidx{b}{h}")
                nc.vector.tensor_tensor(
                    out=fi[:], in0=tbl_f[:], in1=fb[:].to_broadcast([P, NROW_F]),
                    op=mybir.AluOpType.add,
                )
                fidx.append(fi)

        images = t["images"]
        out = t["out"]

        # --- fine branch: 8 groups of 128 patches ---
        # HW indirect DMA consumes ONE offset per destination partition, so each
        # (c,dy) row-column is its own gather instruction writing a 16-elem slice.
        for g in range(IPC * 2):
            b, h = divmod(g, 2)
            gt = gf_pool.tile([P, KF], FDT)
            if no_gather:
                nc.sync.dma_start(
                    gt[:],
                    images.ap().rearrange("(a f) one -> a (f one)", f=KF)[g * P:(g + 1) * P, :],
                )
            else:
                for j in range(NROW_F):
                    nc.gpsimd.indirect_dma_start(
                        out=gt[:, j * FP:(j + 1) * FP], out_offset=None, in_=images[:],
                        in_offset=bass.IndirectOffsetOnAxis(ap=fidx[g][:, j:j + 1], axis=0),
                    )
            if gather_only:
                nc.sync.dma_start(
                    out[b * (NF + NCO) + h * P:b * (NF + NCO) + (h + 1) * P, :],
                    gt[:, :D],
                )
                continue
            psum = ps_f.tile([P, D], FDT)
            lts = []
            for k in range(NKF):
                tp = ps_tp.tile([P, P], FDT, tag="tp")
                nc.tensor.transpose(
                    out=tp[:], in_=gt[:, k * P:(k + 1) * P], identity=identity[:]
                )
                lt = lt_pool.tile([P, P], FDT, tag="lt")
                nc.vector.tensor_copy(lt[:], tp[:])
                lts.append(lt)
            for k in range(NKF):
                wk = wf_pool.tile([P, D], FDT, tag="wf")
                nc.sync.dma_start(wk[:], t["wfT"][k * P:(k + 1) * P, :])
                nc.tensor.matmul(
                    out=psum[:], lhsT=lts[k][:], rhs=wk[:],
                    start=(k == 0), stop=(k == NKF - 1),
                )
            ob = ob_pool.tile([P, D], FDT, tag="ob")
            nc.vector.tensor_tensor(
                out=ob[:], in0=psum[:], in1=bias_f[:], op=mybir.AluOpType.add
            )
            nc.sync.dma_start(out[b * (NF + NCO) + h * P:b * (NF + NCO) + (h + 1) * P, :], ob[:])

        # --- coarse branch: one group of 128 patches, 96 k-chunks ---
        psum_c = None if gather_only else ps_c.tile([P, D], FDT)
        for cc in range(NCHUNK_C):
            gt = gc_pool.tile([P, CJ * CP], FDT)
            if no_gather:
                nc.sync.dma_start(
                    gt[:],
                    images.ap().rearrange("(a f) one -> a (f one)", f=CJ * CP)[cc * P:(cc + 1) * P, :],
                )
            else:
                for j in range(CJ):
                    nc.gpsimd.indirect_dma_start(
                        out=gt[:, j * CP:(j + 1) * CP], out_offset=None, in_=images[:],
                        in_offset=bass.IndirectOffsetOnAxis(
                            ap=cidx[:, cc * CJ + j:cc * CJ + j + 1], axis=0
                        ),
                    )
            if gather_only:
                nc.sync.dma_start(out[cc * P:(cc + 1) * P, :], gt[:, :D])
                continue
            lts = []
            for kk in range(KPC):
                tp = ps_tp.tile([P, P], FDT, tag="tp")
                nc.tensor.transpose(
                    out=tp[:], in_=gt[:, kk * P:(kk + 1) * P], identity=identity[:]
                )
                lt = lt_pool.tile([P, P], FDT, tag="lt")
                nc.vector.tensor_copy(lt[:], tp[:])
                lts.append(lt)
            for kk in range(KPC):
                k = cc * KPC + kk
                wk = wc_pool.tile([P, D], FDT, tag="wc")
                nc.sync.dma_start(wk[:], t["wcT"][k * P:(k + 1) * P, :])
                nc.tensor.matmul(
                    out=psum_c[:], lhsT=lts[kk][:], rhs=wk[:],
                    start=(k == 0), stop=(k == NKC - 1),
                )
        if not gather_only:
            oc = ob_pool.tile([P, D], FDT, tag="oc")
            nc.vector.tensor_tensor(
                out=oc[:], in0=psum_c[:], in1=bias_c[:], op=mybir.AluOpType.add
            )
            for b in range(IPC):
                nc.sync.dma_start(
                    out[b * (NF + NCO) + NF:b * (NF + NCO) + NF + NCO, :],
                    oc[b * NCO:(b + 1) * NCO, :],
                )


def build(reps: int = 1):
    nc = bacc.Bacc("TRN2", target_bir_lowering=False, debug=False)
    t = {
        "images": nc.dram_tensor("images", [NFLAT, 1], FDT, kind="ExternalInput"),
        "fine_xy": nc.dram_tensor("fine_xy", [IPC, NF, 2], IDT, kind="ExternalInput"),
        "coarse_xy": nc.dram_tensor("coarse_xy", [IPC, NCO, 2], IDT, kind="ExternalInput"),
        "wfT": nc.dram_tensor("wfT", [KF, D], FDT, kind="ExternalInput"),
        "wcT": nc.dram_tensor("wcT", [KC, D], FDT, kind="ExternalInput"),
        "bias_f": nc.dram_tensor("bias_f", [P, D], FDT, kind="ExternalInput"),
        "bias_c": nc.dram_tensor("bias_c", [P, D], FDT, kind="ExternalInput"),
        "tbl_f": nc.dram_tensor("tbl_f", [P, NROW_F], IDT, kind="ExternalInput"),
        "tbl_c": nc.dram_tensor("tbl_c", [P, NROW_C], IDT, kind="ExternalInput"),
        "out": nc.dram_tensor("out", [IPC * (NF + NCO), D], FDT, kind="ExternalOutput"),
    }
    with tile.TileContext(nc) as tc:
        for _ in range(reps):
            _emit(nc, tc, t)
    nc.compile()
    return nc


def host_tables():
    jf = np.arange(NROW_F)
    tbl_f = ((jf // FP) * H * W + (jf % FP) * W).astype(np.int32)
    tbl_f = np.repeat(tbl_f[None, :], P, axis=0)
    pc = np.arange(P)[:, None]
    jc = np.arange(NROW_C)[None, :]
    tbl_c = ((pc // NCO) * CHW + (jc // CP) * H * W + (jc % CP) * W).astype(np.int32)
    tbl_c = np.ascontiguousarray(tbl_c)
    return tbl_f, tbl_c


def make_in_maps(images, W_fine, b_fine, W_coarse, b_coarse, fine_xy, coarse_xy):
    images = np.asarray(images, dtype=np.float32)
    fine_xy = np.asarray(fine_xy, dtype=np.int32)
    coarse_xy = np.asarray(coarse_xy, dtype=np.int32)
    wfT = np.ascontiguousarray(np.asarray(W_fine, dtype=np.float32).T)
    wcT = np.ascontiguousarray(np.asarray(W_coarse, dtype=np.float32).T)
    bias_f = np.ascontiguousarray(np.repeat(np.asarray(b_fine, np.float32)[None, :], P, axis=0))
    bias_c = np.ascontiguousarray(np.repeat(np.asarray(b_coarse, np.float32)[None, :], P, axis=0))
    tbl_f, tbl_c = host_tables()
    in_maps = []
    for c in range(NCORES):
        sl = slice(c * IPC, (c + 1) * IPC)
        in_maps.append({
            "images": np.ascontiguousarray(images[sl]).reshape(NFLAT, 1),
            "fine_xy": np.ascontiguousarray(fine_xy[sl]),
            "coarse_xy": np.ascontiguousarray(coarse_xy[sl]),
            "wfT": wfT, "wcT": wcT,
            "bias_f": bias_f, "bias_c": bias_c,
            "tbl_f": tbl_f, "tbl_c": tbl_c,
        })
    return in_maps


_NC_CACHE = []


def _get_nc():
    if not _NC_CACHE:
        _NC_CACHE.append(build())
    return _NC_CACHE[0]


def run(inputs: dict, trace: bool = False):
    nc = _get_nc()
    in_maps = make_in_maps(**inputs)
    res = run_bass_kernel_spmd(nc, in_maps, list(range(NCORES)), trace=trace)
    outs = [
        np.asarray(res.results[c]["out"]).reshape(IPC, NF + NCO, D)
        for c in range(NCORES)
    ]
    return np.concatenate(outs, axis=0), res


def kernel(**inputs) -> np.ndarray:
    out, _ = run(inputs, trace=False)
    return out



# revision 4
# speedup vs baseline: 3.3291x; 3.2142x over previous
"""Trainium2 Bass kernel for CustomPatchEmbedding (ragged patch gather + two projections).

Strategy (data-parallel over batch, 8 cores x 4 images):
  - Patch pixel rows are gathered straight from HBM images via SWDGE
    indirect DMA. One gather INSTRUCTION covers a whole 128-patch tile
    (2D offset AP [128, nrows], 3D dest AP [128, nrows, px]): the 994ns
    per-instruction SWDGE fixed cost is amortized over 6144 descriptors.
  - Gather indices are computed on-chip from the xy tensors (shift/add on
    DVE) plus small constant offset tables supplied as inputs.
  - Gathered f32 tiles are converted to bf16 on the scalar engine; PE
    transposes 128-feature chunks (bf16, 1 cycle/row) into PSUM slices,
    DVE copies them back as bf16 lhsT tiles, and PE accumulates
    lhsT.T @ W^T (bf16, 1 cycle/row vs 4 for f32) into PSUM.
  - Weights are supplied bf16 from the host and loaded with one packed
    DMA each ([K,D] -> [128, (K/128)*D]); bias is added from a
    partition-replicated f32 tile; results DMA to DRAM in f32.

kernel(**inputs) takes the FULL unsharded inputs and returns (32, 288, 256) f32.
"""
import os
import sys
import numpy as np

sys.path.insert(0, "/opt/trn_rl_repo")

import ml_dtypes
import concourse.bass as bass
import concourse.bacc as bacc
import concourse.mybir as mybir
import concourse.tile as tile
from concourse.masks import make_identity
from concourse.bass_utils import run_bass_kernel_spmd
from contextlib import ExitStack

# Problem constants (hardcoded per spec).
B, C, H, W = 32, 3, 512, 512
FP, CP = 16, 64
NF, NCO = 256, 32
D = 256
NCORES = 8
IPC = B // NCORES              # images per core
CHW = C * H * W                # 786432, per-image flat element count
NFLAT = IPC * CHW              # flat image elements per core
KF = C * FP * FP               # 768  fine features
KC = C * CP * CP               # 12288 coarse features
NROW_F = C * FP                # 48 gather rows per fine patch (c,dy)
NROW_C = C * CP                # 192 gather rows per coarse patch
P = 128

FDT = mybir.dt.float32
BDT = mybir.dt.bfloat16
IDT = mybir.dt.int32

NKF = KF // P                  # 6 fine k-chunks of 128
NKC = KC // P                  # 96 coarse k-chunks of 128
# Coarse gather is split into 4 column-chunks of the [128, 192] index tile,
# keeping each gather instruction at 128*48 = 6144 descriptors.
CJ = 48                        # idx columns per coarse gather chunk
NCHUNK_C = NROW_C // CJ        # 4 chunks
KPC = CJ * CP // P             # k-chunks (of 128) per coarse gather chunk = 24
TPG = 3                        # transposes batched per PSUM tile / DVE copy

# "batch" = one SWDGE instruction per gather tile; "safe" = per-row fallback.
BATCH_GATHER = os.environ.get("KGATHER", "batch") == "batch"


def _emit(nc, tc, t):
    """Emit the per-core Tile program. `t` maps tensor name -> dram handle."""
    with ExitStack() as ctx:
        const = ctx.enter_context(tc.tile_pool(name="const", bufs=1))
        small = ctx.enter_context(tc.tile_pool(name="small", bufs=1))
        gf_pool = ctx.enter_context(tc.tile_pool(name="gf", bufs=3))
        gf16_pool = ctx.enter_context(tc.tile_pool(name="gf16", bufs=3))
        gc_pool = ctx.enter_context(tc.tile_pool(name="gc", bufs=2))
        gc16_pool = ctx.enter_context(tc.tile_pool(name="gc16", bufs=2))
        lt_pool = ctx.enter_context(tc.tile_pool(name="lt", bufs=6))
        ob_pool = ctx.enter_context(tc.tile_pool(name="ob", bufs=3))
        ps_tp = ctx.enter_context(tc.tile_pool(name="ps_tp", bufs=4, space="PSUM"))
        ps_f = ctx.enter_context(tc.tile_pool(name="ps_f", bufs=2, space="PSUM"))
        ps_c = ctx.enter_context(tc.tile_pool(name="ps_c", bufs=1, space="PSUM"))

        # --- weights: packed bf16 loads, resident in SBUF ---
        wf16 = const.tile([P, NKF * D], BDT)
        nc.sync.dma_start(
            wf16[:].rearrange("p (a d) -> p a d", d=D),
            t["wfT"].ap().rearrange("(a p) d -> p a d", p=P),
        )
        wc16 = const.tile([P, NKC * D], BDT)
        nc.sync.dma_start(
            wc16[:].rearrange("p (a d) -> p a d", d=D),
            t["wcT"].ap().rearrange("(a p) d -> p a d", p=P),
        )

        # --- constants ---
        identity = const.tile([P, P], BDT)
        make_identity(nc, identity[:])
        tbl_f = const.tile([P, NROW_F], IDT)
        nc.sync.dma_start(tbl_f[:], t["tbl_f"][:])
        tbl_c = const.tile([P, NROW_C], IDT)
        nc.sync.dma_start(tbl_c[:], t["tbl_c"][:])
        bias_f = const.tile([P, D], FDT)
        nc.sync.dma_start(bias_f[:], t["bias_f"][:])
        bias_c = const.tile([P, D], FDT)
        nc.sync.dma_start(bias_c[:], t["bias_c"][:])

        # --- gather indices ---
        # coarse: one [128, 192] tile; partition p = (img, patch), col j = (c, dy)
        cxy = small.tile([P, 2], IDT)
        nc.sync.dma_start(cxy[:], t["coarse_xy"].ap().rearrange("b n two -> (b n) two"))
        cbase = small.tile([P, 1], IDT)
        nc.vector.tensor_scalar(
            out=cbase[:], in0=cxy[:, 1:2], scalar1=9, scalar2=None,
            op0=mybir.AluOpType.logical_shift_left,
        )
        nc.vector.tensor_tensor(
            out=cbase[:], in0=cbase[:], in1=cxy[:, 0:1], op=mybir.AluOpType.add
        )
        cidx = small.tile([P, NROW_C], IDT)
        nc.vector.tensor_tensor(
            out=cidx[:], in0=tbl_c[:], in1=cbase[:].to_broadcast([P, NROW_C]),
            op=mybir.AluOpType.add,
        )

        # fine: per (img b, half h) a [128, 48] tile
        fidx = []
        for b in range(IPC):
            for h in range(2):
                fxy = small.tile([P, 2], IDT, tag="fxy")
                nc.sync.dma_start(fxy[:], t["fine_xy"][b, h * P:(h + 1) * P, :])
                fb = small.tile([P, 1], IDT, tag="fb")
                nc.vector.tensor_scalar(
                    out=fb[:], in0=fxy[:, 1:2], scalar1=9, scalar2=None,
                    op0=mybir.AluOpType.logical_shift_left,
                )
                nc.vector.tensor_tensor(
                    out=fb[:], in0=fb[:], in1=fxy[:, 0:1], op=mybir.AluOpType.add
                )
                nc.vector.tensor_scalar(
                    out=fb[:], in0=fb[:], scalar1=b * CHW, scalar2=None,
                    op0=mybir.AluOpType.add,
                )
                fi = small.tile([P, NROW_F], IDT, tag=f"fidx{b}{h}")
                nc.vector.tensor_tensor(
                    out=fi[:], in0=tbl_f[:], in1=fb[:].to_broadcast([P, NROW_F]),
                    op=mybir.AluOpType.add,
                )
                fidx.append(fi)

        images = t["images"]
        out = t["out"]

        def gather(gt, idx_ap, ncols, px):
            """Gather ncols*px-wide [128, ncols*px] tile; idx_ap [128, ncols]."""
            if BATCH_GATHER:
                nc.gpsimd.indirect_dma_start(
                    out=gt[:].rearrange("p (j k) -> p j k", k=px),
                    out_offset=None, in_=images[:],
                    in_offset=bass.IndirectOffsetOnAxis(ap=idx_ap, axis=0),
                )
            else:
                for j in range(ncols):
                    nc.gpsimd.indirect_dma_start(
                        out=gt[:, j * px:(j + 1) * px], out_offset=None,
                        in_=images[:],
                        in_offset=bass.IndirectOffsetOnAxis(
                            ap=idx_ap[:, j:j + 1], axis=0
                        ),
                    )

        def project(gt16, nk, psum, wtile, kbase, nktot):
            """Transpose nk 128-chunks of gt16 and accumulate into psum."""
            for c0 in range(0, nk, TPG):
                cn = min(TPG, nk - c0)
                tp = ps_tp.tile([P, TPG * P], BDT, tag="tp")
                for c in range(c0, c0 + cn):
                    nc.tensor.matmul(
                        out=tp[:, (c - c0) * P:(c - c0 + 1) * P],
                        lhsT=gt16[:, c * P:(c + 1) * P], rhs=identity[:],
                        start=True, stop=True, is_transpose=True,
                        skip_group_check=True,
                    )
                lt = lt_pool.tile([P, TPG * P], BDT, tag="lt")
                nc.vector.tensor_copy(lt[:, :cn * P], tp[:, :cn * P])
                for c in range(c0, c0 + cn):
                    k = kbase + c
                    nc.tensor.matmul(
                        out=psum[:], lhsT=lt[:, (c - c0) * P:(c - c0 + 1) * P],
                        rhs=wtile[:, k * D:(k + 1) * D],
                        start=(k == 0), stop=(k == nktot - 1),
                    )

        # --- fine branch: 8 groups of 128 patches ---
        for g in range(IPC * 2):
            b, h = divmod(g, 2)
            gt = gf_pool.tile([P, KF], FDT)
            gather(gt, fidx[g][:, :], NROW_F, FP)
            gt16 = gf16_pool.tile([P, KF], BDT)
            nc.scalar.copy(gt16[:], gt[:])
            psum = ps_f.tile([P, D], FDT)
            project(gt16, NKF, psum, wf16, 0, NKF)
            ob = ob_pool.tile([P, D], FDT, tag="ob")
            nc.vector.tensor_tensor(
                out=ob[:], in0=psum[:], in1=bias_f[:], op=mybir.AluOpType.add
            )
            nc.sync.dma_start(
                out[b * (NF + NCO) + h * P:b * (NF + NCO) + (h + 1) * P, :], ob[:]
            )

        # --- coarse branch: one group of 128 patches, 4 gather chunks ---
        psum_c = ps_c.tile([P, D], FDT)
        for cc in range(NCHUNK_C):
            gt = gc_pool.tile([P, CJ * CP], FDT)
            gather(gt, cidx[:, cc * CJ:(cc + 1) * CJ], CJ, CP)
            gt16 = gc16_pool.tile([P, CJ * CP], BDT)
            nc.scalar.copy(gt16[:], gt[:])
            project(gt16, KPC, psum_c, wc16, cc * KPC, NKC)
        oc = ob_pool.tile([P, D], FDT, tag="oc")
        nc.vector.tensor_tensor(
            out=oc[:], in0=psum_c[:], in1=bias_c[:], op=mybir.AluOpType.add
        )
        for b in range(IPC):
            nc.sync.dma_start(
                out[b * (NF + NCO) + NF:b * (NF + NCO) + NF + NCO, :],
                oc[b * NCO:(b + 1) * NCO, :],
            )


def build(reps: int = 1):
    nc = bacc.Bacc("TRN2", target_bir_lowering=False, debug=False)
    t = {
        "images": nc.dram_tensor("images", [NFLAT, 1], FDT, kind="ExternalInput"),
        "fine_xy": nc.dram_tensor("fine_xy", [IPC, NF, 2], IDT, kind="ExternalInput"),
        "coarse_xy": nc.dram_tensor("coarse_xy", [IPC, NCO, 2], IDT, kind="ExternalInput"),
        "wfT": nc.dram_tensor("wfT", [KF, D], BDT, kind="ExternalInput"),
        "wcT": nc.dram_tensor("wcT", [KC, D], BDT, kind="ExternalInput"),
        "bias_f": nc.dram_tensor("bias_f", [P, D], FDT, kind="ExternalInput"),
        "bias_c": nc.dram_tensor("bias_c", [P, D], FDT, kind="ExternalInput"),
        "tbl_f": nc.dram_tensor("tbl_f", [P, NROW_F], IDT, kind="ExternalInput"),
        "tbl_c": nc.dram_tensor("tbl_c", [P, NROW_C], IDT, kind="ExternalInput"),
        "out": nc.dram_tensor("out", [IPC * (NF + NCO), D], FDT, kind="ExternalOutput"),
    }
    with tile.TileContext(nc) as tc:
        for _ in range(reps):
            _emit(nc, tc, t)
    nc.compile()
    return nc


def host_tables():
    jf = np.arange(NROW_F)
    tbl_f = ((jf // FP) * H * W + (jf % FP) * W).astype(np.int32)
    tbl_f = np.repeat(tbl_f[None, :], P, axis=0)
    pc = np.arange(P)[:, None]
    jc = np.arange(NROW_C)[None, :]
    tbl_c = ((pc // NCO) * CHW + (jc // CP) * H * W + (jc % CP) * W).astype(np.int32)
    tbl_c = np.ascontiguousarray(tbl_c)
    return tbl_f, tbl_c


def make_in_maps(images, W_fine, b_fine, W_coarse, b_coarse, fine_xy, coarse_xy):
    images = np.asarray(images, dtype=np.float32)
    fine_xy = np.asarray(fine_xy, dtype=np.int32)
    coarse_xy = np.asarray(coarse_xy, dtype=np.int32)
    wfT = np.ascontiguousarray(
        np.asarray(W_fine, dtype=np.float32).T.astype(ml_dtypes.bfloat16)
    )
    wcT = np.ascontiguousarray(
        np.asarray(W_coarse, dtype=np.float32).T.astype(ml_dtypes.bfloat16)
    )
    bias_f = np.ascontiguousarray(np.repeat(np.asarray(b_fine, np.float32)[None, :], P, axis=0))
    bias_c = np.ascontiguousarray(np.repeat(np.asarray(b_coarse, np.float32)[None, :], P, axis=0))
    tbl_f, tbl_c = host_tables()
    in_maps = []
    for c in range(NCORES):
        sl = slice(c * IPC, (c + 1) * IPC)
        in_maps.append({
            "images": np.ascontiguousarray(images[sl]).reshape(NFLAT, 1),
            "fine_xy": np.ascontiguousarray(fine_xy[sl]),
            "coarse_xy": np.ascontiguousarray(coarse_xy[sl]),
            "wfT": wfT, "wcT": wcT,
            "bias_f": bias_f, "bias_c": bias_c,
            "tbl_f": tbl_f, "tbl_c": tbl_c,
        })
    return in_maps


_NC_CACHE = []


def _get_nc():
    if not _NC_CACHE:
        _NC_CACHE.append(build())
    return _NC_CACHE[0]


def run(inputs: dict, trace: bool = False):
    nc = _get_nc()
    in_maps = make_in_maps(**inputs)
    res = run_bass_kernel_spmd(nc, in_maps, list(range(NCORES)), trace=trace)
    outs = [
        np.asarray(res.results[c]["out"]).reshape(IPC, NF + NCO, D)
        for c in range(NCORES)
    ]
    return np.concatenate(outs, axis=0), res


def kernel(**inputs) -> np.ndarray:
    out, _ = run(inputs, trace=False)
    return out
